# revision 24
# baseline (speedup 1.0000x reference)
"""Trainium2 Bass kernel for a pre-norm transformer block with dilated
windowed causal attention (B=2, L=2048, D=512, H=8, DIL=2, WIN=256,
HIDDEN=2048).

Sharding: 8 cores = batch(2) x sequence-chunk(4 x 512 tokens). Each core
receives its 512-token chunk plus a 256-token halo (keys/values only) and
computes the full block for its tokens; no collectives.

v2 design (vs baseline):
- QKV / out-proj / FFN matmuls run in fp8e4 with DoubleRow perf mode
  (2 k-tiles per matmul). Weights are pre-scaled x16 host-side; the 1/16
  unscale is folded into the PSUM->SBUF copies (or the softmax reciprocal
  for the V path).
- The dilated/causal band mask is applied by PE accumulation: a second
  matmul adds -240 * (1-mask) into the score PSUM, so exp() reads masked
  scores directly; no separate elementwise mask pass.
- One exp per (head-pair, head-half) chain over a [128,1024] PSUM tile
  spanning 2 banks.
- Softmax normalization: the ones-row in V gives the denominator in the
  PV PSUM; reciprocal reads it straight from PSUM; an esel matmul (scaled
  1/256 to fold both x16 scales) broadcasts it; the normalize multiply
  reads PV PSUM + rb PSUM directly and writes oT in fp8.
- Engine balance: DVE = LN stats + QK copies + norm + residuals;
  ACT = exp/gelu/V-copies/FFN2-unscale; Pool = LN applies + transpose
  copies.
"""
import os
import sys

os.environ.setdefault("MYCRO_LOCAL_CACHE", "1")
if "/opt/trn_rl_repo" not in sys.path:
    sys.path.insert(0, "/opt/trn_rl_repo")

import numpy as np

B, L, D, H = 2, 2048, 512, 8
HD = D // H
HIDDEN = 4 * D
P = 128
CH = 512            # own tokens per core
HALO = 256
T = CH + HALO       # 768
NCORES = 8
EPS = 1e-5
SQ = CH // 2        # 256 queries per parity stream
SCALE = 1.0 / 8.0   # 1/sqrt(HD)
WS = 16.0           # fp8 weight pre-scale
RS = 1.0 / WS

NT = T // P         # 6
NO = CH // P        # 4
ND = D // P         # 4
NHID = HIDDEN // P  # 16

_nc = {}
LAST_EXEC_NS = None
LAST_RESULTS = None


def _body(ctx, tc, I, y, has_bias):
    import concourse.bass as bass  # noqa: F401
    from concourse import mybir
    from concourse.masks import make_identity

    nc = tc.nc
    f32 = mybir.dt.float32
    f32r = mybir.dt.float32r
    bf16 = mybir.dt.bfloat16
    fp8 = mybir.dt.float8e4
    AF = mybir.ActivationFunctionType
    OP = mybir.AluOpType
    DR = mybir.MatmulPerfMode.DoubleRow

    consts = ctx.enter_context(tc.tile_pool(name="consts", bufs=1))
    big = ctx.enter_context(tc.tile_pool(name="big", bufs=1))
    work = ctx.enter_context(tc.tile_pool(name="work", bufs=4))
    pexp = ctx.enter_context(tc.tile_pool(name="pexp", bufs=4))
    # PSUM: pa_s 2x2 banks + pa_o 2x1 + pmm 2x1 = 8 banks total
    pa_s = ctx.enter_context(tc.tile_pool(name="pa_s", bufs=2, space="PSUM"))
    pa_o = ctx.enter_context(tc.tile_pool(name="pa_o", bufs=3, space="PSUM"))
    pmm = ctx.enter_context(tc.tile_pool(name="pmm", bufs=1, space="PSUM"))

    mm = nc.tensor.matmul

    def bcast(ap, p=P):
        return bass.AP(tensor=ap.tensor, offset=ap.offset,
                       ap=[[0, p]] + [list(d) for d in ap.ap])

    # ---------- constants ----------
    ident = consts.tile([P, P], bf16, tag="ident")
    make_identity(nc, ident)
    epst = consts.tile([P, 1], f32, tag="eps")
    nc.vector.memset(epst, EPS)
    # esel: one-hot (x 1/256) selector that broadcasts the 4 stream/headhalf
    # denominator rows (0=(s0,h0) 1=(s0,h1) 2=(s1,h0) 3=(s1,h1)) to the
    # right output partitions.
    esel = consts.tile([97, P], f32, tag="esel")
    nc.vector.memset(esel, 0.0)
    nc.vector.memset(esel[0:1, 0:64], 1.0 / 256.0)    # (stp0, hh0)
    nc.vector.memset(esel[32:33, 64:128], 1.0 / 256.0)  # (stp0, hh1)
    nc.vector.memset(esel[64:65, 0:64], 1.0 / 256.0)  # (stp1, hh0)
    nc.vector.memset(esel[96:97, 64:128], 1.0 / 256.0)  # (stp1, hh1)
    # denominator tiles (one per head pair); the never-written parts stay
    # at 1e30 so their reciprocal is ~0 and drops out of the esel sum.
    dens, rdens = [], []
    for hp in range(4):
        dn = consts.tile([97, 2 * SQ], f32, tag=f"den{hp}")
        nc.vector.memset(dn, 1e30)
        dens.append(dn)
        rd = consts.tile([97, 2 * SQ], f32, tag=f"rden{hp}")
        rdens.append(rd)
    # V ones rows (col 64 of each head slot)
    v_sb = big.tile([P, 6, H, 65], bf16, tag="v")
    for s in range(6):
        nc.vector.memset(v_sb[:, s, :, 64:65], 1.0)

    # PE warm-up while x streams in: keeps the activity window busy so the
    # PE p-state is at full clock when real work starts.
    junk = pmm.tile([P, D], f32, tag="ps")
    for _ in range(30):
        mm(junk[:, 0:P], ident, ident, start=True, stop=True)

    # preload the EXP and GELU activation tables during the idle start
    tdum = work.tile([P, 1], f32, tag="lnr")
    nc.scalar.activation(tdum, epst, AF.Exp, scale=1.0)
    tdum2 = work.tile([P, 1], f32, tag="lnr2")
    nc.scalar.activation(tdum2, epst, AF.Gelu, scale=1.0)

    # ---------- input DMAs (sync queue, priority order) ----------
    pari = consts.tile([P, P], bf16, tag="pari")
    nc.sync.dma_start(out=pari, in_=I["pari"])
    x_sb = big.tile([P, NT, D], bf16, tag="x")
    for c0 in (2, 4, 0):
        nc.sync.dma_start(out=x_sb[:, c0:c0 + 2, :], in_=I["xc"][:, c0:c0 + 2, :])
    wqkv_sb = big.tile([P, ND, 3 * D], fp8, tag="wqkv")
    nc.sync.dma_start(out=wqkv_sb, in_=I["wqkvT"])
    minv_sb = consts.tile([P, 4 * SQ], bf16, tag="minv")
    nc.sync.dma_start(out=minv_sb, in_=I["minv"])
    wo_sb = big.tile([P, ND, D], fp8, tag="wo")
    nc.sync.dma_start(out=wo_sb, in_=I["woT"])
    w1_sb = big.tile([P, ND, HIDDEN], fp8, tag="w1")
    nc.sync.dma_start(out=w1_sb, in_=I["w1T"])
    w2_sb = big.tile([P, NHID, D], fp8, tag="w2")
    nc.sync.dma_start(out=w2_sb, in_=I["w2T"])
    # full-precision own tokens for the residual path (needed late)
    xo_sb = big.tile([P, NO, D], f32, tag="xo")
    nc.sync.dma_start(out=xo_sb, in_=I["xo"])
    if has_bias:
        bq_sb = consts.tile([P, 4], f32, tag="bq")
        nc.sync.dma_start(out=bq_sb, in_=I["bq"])
        bk_sb = consts.tile([P, 4], f32, tag="bk")
        nc.sync.dma_start(out=bk_sb, in_=I["bk"])
        b1_sb = consts.tile([P, NHID], f32, tag="b1")
        nc.sync.dma_start(out=b1_sb, in_=I["b1"])
        bv16_sb = consts.tile([1, D], bf16, tag="bv16")
        nc.sync.dma_start(out=bv16_sb, in_=I["bv16"])
        ones1 = consts.tile([1, P], bf16, tag="ones1")
        nc.vector.memset(ones1, 1.0)
        bo_sb = consts.tile([P, D], f32, tag="bo")
        nc.gpsimd.dma_start(out=bo_sb, in_=bcast(I["bo"]))
        b2_sb = consts.tile([P, D], f32, tag="b2")
        nc.gpsimd.dma_start(out=b2_sb, in_=bcast(I["b2"]))

    # ---------- LN1 (stats on DVE, apply on DVE) + transpose + V ----------
    # xT is parity-blocked: col = stp*384 + stream_index (stream = token//2)
    # V-slot matmuls are woven in as their two source chunks complete.
    SL = T // 2  # 384
    xhat = big.tile([P, NT, D], bf16, tag="xhat")
    xT = big.tile([P, ND, 2, SL], fp8, tag="xT")
    qT = big.tile([P, 4, CH], bf16, tag="qT")
    kT = big.tile([P, 4, T], bf16, tag="kT")
    xTf = xT.rearrange("p a s c -> p a (s c)")
    for j in (2, 3, 4, 5, 0, 1):
        st = work.tile([P, 6], f32, tag="bnst")
        nc.vector.bn_stats(st, x_sb[:, j, :])
        with tc.high_priority(offset=40):
            mv = work.tile([P, 2], f32, tag="bnmv")
            nc.vector.bn_aggr(mv, st)
            r = work.tile([P, 1], f32, tag="lnr")
            nc.scalar.activation(r, mv[:, 1:2], AF.Sqrt, bias=epst, scale=1.0)
            r2 = work.tile([P, 1], f32, tag="lnr2")
            nc.vector.reciprocal(r2, r)
            nc.vector.tensor_scalar(
                out=xhat[:, j, :], in0=x_sb[:, j, :],
                scalar1=mv[:, 0:1], scalar2=r2,
                op0=OP.subtract, op1=OP.mult,
            )
            # transpose with a parity-permutation rhs: output comes out
            # parity-blocked [evens | odds], so the copy is unit-stride
            if j % 2 == 0:
                pt = pmm.tile([P, 2 * D], bf16, tag="ps")
            else:
                pt = pa_o.tile([P, 2 * D], bf16, tag="po")
            for dt_ in range(ND):
                nc.tensor.transpose(pt[:, dt_ * P:(dt_ + 1) * P],
                                    xhat[:, j, dt_ * P:(dt_ + 1) * P], pari)
            nc.scalar.copy(
                xT[:, :, :, j * 64:(j + 1) * 64],
                pt[:, 0:D].rearrange("p (a s c) -> p a s c", a=ND, s=2))
        if j % 2 == 1:
            # V slots for stream block i = (j-1)//2 (both parities)
            i = (j - 1) // 2
            for stp in range(2):
                s = stp * 3 + i
                ps = pa_o.tile([P, CH], f32, tag="po")
                if has_bias:
                    mm(ps, ones1, bv16_sb, start=True, stop=False)
                for dp in range(2):
                    mm(ps, xT[:, 2 * dp:2 * dp + 2, stp, i * P:(i + 1) * P],
                       wqkv_sb[:, 2 * dp:2 * dp + 2, 2 * D:3 * D],
                       start=(dp == 0 and not has_bias), stop=(dp == 1),
                       perf_mode=DR)
                nc.scalar.copy(
                    v_sb[:, s, :, 0:64],
                    ps.rearrange("p (h c) -> p h c", h=H))

    if has_bias:
        for tt in range(NO):
            nc.gpsimd.tensor_add(xo_sb[:, tt, :], xo_sb[:, tt, :], bo_sb)

    # ---------- Q/K (fp8 DoubleRow; weights x16, unscale 1/16 on copy) ----
    # qT cols are parity-blocked: stp*256 + rel_own_query
    # kT cols are parity-blocked: stp*384 + stream_key
    for og in range(2):   # pairs of ot
        ps = pa_s.tile([P, 2, CH], f32, tag="ps_s")
        for oi in range(2):
            ot = og * 2 + oi
            for stp in range(2):
                for dp in range(2):
                    mm(ps[:, oi, stp * SQ:(stp + 1) * SQ],
                       wqkv_sb[:, 2 * dp:2 * dp + 2, ot * P:(ot + 1) * P],
                       xT[:, 2 * dp:2 * dp + 2, stp, SL - SQ:],
                       start=(dp == 0 and stp == 0), stop=(dp == 1 and stp == 1),
                       perf_mode=DR)
        if has_bias:
            for oi in range(2):
                ot = og * 2 + oi
                nc.vector.tensor_scalar(
                    out=qT[:, ot, :], in0=ps[:, oi, :],
                    scalar1=RS, scalar2=bq_sb[:, ot:ot + 1],
                    op0=OP.mult, op1=OP.add)
        else:
            nc.vector.tensor_scalar(
                out=qT[:, og * 2:og * 2 + 2, :].rearrange("p a b -> p (a b)"),
                in0=ps.rearrange("p a b -> p (a b)"),
                scalar1=RS, scalar2=None, op0=OP.mult)

    for ot in range(4):
        ps = pa_s.tile([P, 2, CH], f32, tag="ps_s")
        for dp in range(2):
            mm(ps[:, 0, :], wqkv_sb[:, 2 * dp:2 * dp + 2, (4 + ot) * P:(5 + ot) * P],
               xTf[:, 2 * dp:2 * dp + 2, 0:CH],
               start=(dp == 0), stop=(dp == 1), perf_mode=DR)
        for dp in range(2):
            mm(ps[:, 1, 0:HALO], wqkv_sb[:, 2 * dp:2 * dp + 2, (4 + ot) * P:(5 + ot) * P],
               xTf[:, 2 * dp:2 * dp + 2, CH:],
               start=(dp == 0), stop=(dp == 1), perf_mode=DR)
        if has_bias:
            nc.vector.tensor_scalar(
                out=kT[:, ot, :],
                in0=ps.rearrange("p a b -> p (a b)")[:, 0:T],
                scalar1=RS, scalar2=bk_sb[:, ot:ot + 1],
                op0=OP.mult, op1=OP.add)
        else:
            nc.vector.tensor_scalar(
                out=kT[:, ot, :],
                in0=ps.rearrange("p a b -> p (a b)")[:, 0:T],
                scalar1=RS, scalar2=None, op0=OP.mult)

    # ---------- attention ----------
    # score PSUM layout per chain (2 banks):
    #   bank0 cols 0:512    kt1: [stp0 256 | stp1 256]
    #   bank1 cols 512:1024 kt0: [stp0 128 | stp1 128], kt2: [stp0 | stp1]
    oT = big.tile([P, 4, CH], fp8, tag="oT")

    def emit_S(hp, hh):
        lo = hh * 64
        ps = pa_s.tile([P, 2, 2 * SQ], f32, tag="ps_s")
        first = {0: True, 1: True}
        order = [(1, 0), (1, 1), (0, 0), (0, 1), (2, 0), (2, 1)]
        SL_ = T // 2
        for kt, stp in order:
            k0 = stp * SL_ + kt * P
            q0 = stp * SQ + (0 if kt < 2 else P)
            qw = SQ if kt == 1 else P
            if kt == 1:
                c0 = stp * SQ
                bank = 0
            elif kt == 0:
                c0 = 2 * SQ + stp * P
                bank = 1
            else:
                c0 = 2 * SQ + 2 * P + stp * P
                bank = 1
            pv = ps.rearrange("p a b -> p (a b)")
            mm(pv[:, c0:c0 + qw],
               kT[lo:lo + 64, hp, k0:k0 + P],
               qT[lo:lo + 64, hp, q0:q0 + qw],
               start=first[bank], stop=False)
            first[bank] = False
        # mask accumulation (bank0: kt1 block; bank1: kt0|kt2 blocks)
        pv = ps.rearrange("p a b -> p (a b)")
        mm(pv[:, 0:2 * SQ], ident, minv_sb[:, 0:2 * SQ], start=False, stop=True)
        mm(pv[:, 2 * SQ:4 * SQ], ident, minv_sb[:, 2 * SQ:4 * SQ],
           start=False, stop=True)
        p_sb = pexp.tile([P, 4 * SQ], bf16, tag="p_sb")
        nc.scalar.activation(p_sb, pv, AF.Exp, scale=SCALE)
        return p_sb

    def emit_PV(hp, hh, p_sb, late=False):
        h = 2 * hp + hh
        po = pa_o.tile([P, 2 * SQ], f32, tag="po")
        n = 0
        for stp in range(2):
            qa = stp * SQ
            kt0c = 2 * SQ + stp * P
            kt1c = stp * SQ
            kt2c = 2 * SQ + 2 * P + stp * P
            pieces = [
                (qa, SQ, v_sb[:, stp * 3 + 1, h, :], p_sb[:, kt1c:kt1c + SQ]),
                (qa, P, v_sb[:, stp * 3 + 0, h, :], p_sb[:, kt0c:kt0c + P]),
                (qa + P, P, v_sb[:, stp * 3 + 2, h, :], p_sb[:, kt2c:kt2c + P]),
            ]
            for qc, qn, vv, pp in pieces:
                mm(po[0:65, qc:qc + qn], vv, pp,
                   start=(n == 0), stop=(n == 5))
                n += 1
        # denominator rows (the esel 1/256 folds the V x16 and oT /16)
        dn = dens[hp]
        with tc.high_priority():
            if late:
                # post-chain tail: keep DVE free for recip/muls
                nc.scalar.copy(dn[32 * hh:32 * hh + 1, 0:SQ],
                               po[64:65, 0:SQ])
            else:
                nc.vector.tensor_copy(dn[32 * hh:32 * hh + 1, 0:SQ],
                                      po[64:65, 0:SQ])
            nc.scalar.copy(dn[64 + 32 * hh:65 + 32 * hh, SQ:2 * SQ],
                           po[64:65, SQ:2 * SQ])
        return po

    def emit_norm(hp, po0, po1):
        with tc.high_priority():
            nc.vector.reciprocal_approx_fast(rdens[hp], dens[hp])
            rb = pmm.tile([P, 2 * SQ], f32, tag="ps")
            mm(rb, esel, rdens[hp], start=True, stop=True)
            rb_sb = work.tile([P, 2 * SQ], bf16, tag="rb_sb")
            if hp >= 2:
                nc.scalar.copy(rb_sb, rb)
            else:
                nc.vector.tensor_copy(rb_sb, rb)
            with nc.allow_low_precision(reason="fp8 attention output"):
                for hh, po in ((0, po0), (1, po1)):
                    lo = hh * 64
                    nc.vector.tensor_mul(
                        oT[lo:lo + 64, hp, :].rearrange("p (q s) -> p s q", s=2),
                        po[0:64, :].rearrange("p (s q) -> p s q", s=2),
                        rb_sb[lo:lo + 64, :].rearrange("p (s q) -> p s q", s=2))

    def keepalive(n):
        jt = pa_s.tile([P, 2, CH], f32, tag="ps_s")
        for _ in range(n):
            mm(jt[:, 0, :], ident, minv_sb[:, 0:CH], start=True, stop=True)

    chains = [(hp, hh) for hp in range(4) for hh in range(2)]
    prev = None
    pohist = {}
    for ci, (hp, hh) in enumerate(chains):
        p_sb = emit_S(hp, hh)
        if prev is not None:
            pohist[ci - 1] = emit_PV(prev[0], prev[1], pprev, late=(ci - 1 >= 6))
        if ci >= 3 and ci % 2 == 1:
            nhp = (ci - 3) // 2
            emit_norm(nhp, pohist[2 * nhp], pohist[2 * nhp + 1])
        prev = (hp, hh)
        pprev = p_sb
    pohist[7] = emit_PV(prev[0], prev[1], pprev, late=True)
    keepalive(8)
    emit_norm(3, pohist[6], pohist[7])

    # ---------- out projection + residual (res1 = o @ wo + x) ----------
    res1 = big.tile([P, NO, D], f32, tag="res1")
    xhat2 = big.tile([P, NO, D], bf16, tag="xhat2")
    x2T = big.tile([P, ND, CH], fp8, tag="x2T")
    op_ps = []
    for tt in range(NO):
        ps = pa_o.tile([P, D], f32, tag="po")
        for dp in range(2):
            mm(ps, oT[:, 2 * dp:2 * dp + 2, tt * P:(tt + 1) * P],
               wo_sb[:, 2 * dp:2 * dp + 2, :],
               start=(dp == 0), stop=(dp == 1), perf_mode=DR)
        op_ps.append(ps)
    for tt in range(NO):
        nc.vector.tensor_add(res1[:, tt, :], op_ps[tt], xo_sb[:, tt, :])
        # LN2 for this token block
        st = work.tile([P, 6], f32, tag="bnst")
        nc.vector.bn_stats(st, res1[:, tt, :])
        mv = work.tile([P, 2], f32, tag="bnmv")
        nc.vector.bn_aggr(mv, st)
        r = work.tile([P, 1], f32, tag="lnr")
        nc.scalar.activation(r, mv[:, 1:2], AF.Sqrt, bias=epst, scale=1.0)
        r2 = work.tile([P, 1], f32, tag="lnr2")
        nc.vector.reciprocal(r2, r)
        nc.vector.tensor_scalar(
            out=xhat2[:, tt, :], in0=res1[:, tt, :],
            scalar1=mv[:, 0:1], scalar2=r2,
            op0=OP.subtract, op1=OP.mult,
        )
        if has_bias:
            nc.gpsimd.tensor_add(res1[:, tt, :], res1[:, tt, :], b2_sb)
        if tt % 2 == 0:
            pt = pmm.tile([P, 2 * D], bf16, tag="ps")
        else:
            pt = pa_o.tile([P, 2 * D], bf16, tag="po")
        for dt_ in range(ND):
            nc.tensor.transpose(pt[:, dt_ * P:(dt_ + 1) * P],
                                xhat2[:, tt, dt_ * P:(dt_ + 1) * P], ident)
        nc.scalar.copy(
            x2T[:, :, tt * P:(tt + 1) * P],
            pt[:, 0:D].rearrange("p (a b) -> p a b", a=ND))
        keepalive(4)

    # ---------- FFN1 + gelu (fp8 DoubleRow) ----------
    g_sb = big.tile([P, NHID, CH], fp8, tag="g")
    for hg in range(NHID // 2):
        ps = pa_s.tile([P, 2, CH], f32, tag="ps_s")
        for hi in range(2):
            ht = hg * 2 + hi
            for dp in range(2):
                mm(ps[:, hi, :],
                   w1_sb[:, 2 * dp:2 * dp + 2, ht * P:(ht + 1) * P],
                   x2T[:, 2 * dp:2 * dp + 2, :],
                   start=(dp == 0), stop=(dp == 1), perf_mode=DR)
        if has_bias:
            for hi in range(2):
                ht = hg * 2 + hi
                nc.scalar.activation(g_sb[:, ht, :], ps[:, hi, :], AF.Gelu,
                                     bias=b1_sb[:, ht:ht + 1], scale=RS)
        else:
            nc.scalar.activation(
                g_sb[:, hg * 2:hg * 2 + 2, :].rearrange("p a b -> p (a b)"),
                ps.rearrange("p a b -> p (a b)"), AF.Gelu, scale=RS)

    # ---------- FFN2 + residual + output DMA ----------
    yr = y.rearrange("(j p) d -> p j d", p=P)
    for tt in range(NO):
        ps = pa_o.tile([P, D], f32, tag="po")
        for hp_ in range(NHID // 2):
            mm(ps, g_sb[:, 2 * hp_:2 * hp_ + 2, tt * P:(tt + 1) * P],
               w2_sb[:, 2 * hp_:2 * hp_ + 2, :],
               start=(hp_ == 0), stop=(hp_ == NHID // 2 - 1), perf_mode=DR)
        f2 = work.tile([P, D], f32, tag="f2")
        nc.scalar.activation(f2, ps, AF.Identity, scale=RS)
        nc.vector.tensor_add(res1[:, tt, :], f2, res1[:, tt, :])
        nc.sync.dma_start(out=yr[:, tt, :], in_=res1[:, tt, :])


def _build(has_bias):
    from contextlib import ExitStack

    import concourse.bacc as bacc
    import concourse.tile as tile
    from concourse import mybir

    f32 = mybir.dt.float32
    bf16 = mybir.dt.bfloat16
    fp8 = mybir.dt.float8e4
    nc = bacc.Bacc("TRN2", target_bir_lowering=False, debug=False,
                   enable_asserts=False, num_devices=NCORES)
    I = {}

    def inp(name, shape, dt_):
        I[name] = nc.dram_tensor(name, list(shape), dt_, kind="ExternalInput").ap()

    inp("xc", (P, NT, D), bf16)
    inp("xo", (P, NO, D), f32)
    inp("wqkvT", (P, ND, 3 * D), fp8)
    inp("woT", (P, ND, D), fp8)
    inp("w1T", (P, ND, HIDDEN), fp8)
    inp("w2T", (P, NHID, D), fp8)
    inp("minv", (P, 4 * SQ), bf16)
    inp("pari", (P, P), bf16)
    if has_bias:
        inp("bq", (P, 4), f32)
        inp("bk", (P, 4), f32)
        inp("b1", (P, NHID), f32)
        inp("bv16", (1, D), bf16)
        inp("bo", (D,), f32)
        inp("b2", (D,), f32)
    y = nc.dram_tensor("y", [CH, D], f32, kind="ExternalOutput").ap()

    with tile.TileContext(nc) as tc:
        with ExitStack() as ctx:
            _body(ctx, tc, I, y, has_bias)
    nc.compile()
    return nc


def _host_masks():
    import ml_dtypes
    SL = T // 2
    SW = 128
    sk = np.arange(SL)[:, None]
    sq = np.arange(SL - SQ, SL)[None, :]
    valid = ((sq - sk >= 0) & (sq - sk <= SW)).astype(np.float32)  # [384,256]
    kt0 = valid[0:P, 0:P]            # keys 0:128 x rel-q 0:128
    kt1 = valid[P:2 * P, :]          # keys 128:256 x rel-q 0:256
    kt2 = valid[2 * P:3 * P, P:SQ]   # keys 256:384 x rel-q 128:256
    z128 = np.zeros((P, P), np.float32)

    def inv(m):
        return -240.0 * (1.0 - m)
    m = np.concatenate([inv(kt1), inv(kt1), inv(kt0), inv(kt0),
                        inv(kt2), inv(kt2)], 1)                  # [128,1024]
    m0 = np.concatenate([inv(kt1), inv(kt1), inv(z128), inv(z128),
                         inv(kt2), inv(kt2)], 1)                 # halo invalid
    return (np.ascontiguousarray(m).astype(ml_dtypes.bfloat16),
            np.ascontiguousarray(m0).astype(ml_dtypes.bfloat16))


def get_nc(has_bias):
    if has_bias not in _nc:
        _nc[has_bias] = _build(has_bias)
    return _nc[has_bias]


def _pmaj(a, p=P):
    """[N*p, F...] row-major -> [p, N, F...] partition-major contiguous."""
    n = a.shape[0] // p
    return np.ascontiguousarray(
        a.reshape((n, p) + a.shape[1:]).transpose((1, 0) + tuple(range(2, a.ndim + 1))))


def make_in_maps(inputs):
    import ml_dtypes
    f = np.float32
    f8 = ml_dtypes.float8_e4m3
    bf = ml_dtypes.bfloat16
    x = np.asarray(inputs["x"], f)
    qkv_w = np.asarray(inputs["qkv_w"], f)
    n1w = np.asarray(inputs["norm1_w"], f)
    n1b = np.asarray(inputs["norm1_b"], f)
    wqkv_f = qkv_w * n1w[None, :]
    bqkv = qkv_w @ n1b + np.asarray(inputs["qkv_b"], f)
    wqkvT = _pmaj(np.ascontiguousarray(wqkv_f.T * WS).astype(f8))
    woT = _pmaj(np.ascontiguousarray(
        np.asarray(inputs["out_w"], f).T * WS).astype(f8))
    bo = np.ascontiguousarray(np.asarray(inputs["out_b"], f))

    w1 = np.asarray(inputs["ffn_w1"], f)
    n2w = np.asarray(inputs["norm2_w"], f)
    n2b = np.asarray(inputs["norm2_b"], f)
    w1T = _pmaj(np.ascontiguousarray((w1 * n2w[None, :]).T * WS).astype(f8))
    b1v = w1 @ n2b + np.asarray(inputs["ffn_b1"], f)
    w2T = _pmaj(np.ascontiguousarray(
        np.asarray(inputs["ffn_w2"], f).T * WS).astype(f8))
    b2 = np.ascontiguousarray(np.asarray(inputs["ffn_b2"], f))

    has_bias = bool(
        np.abs(bqkv).max() > 0 or np.abs(bo).max() > 0
        or np.abs(b1v).max() > 0 or np.abs(b2).max() > 0)

    masks, masks0 = _host_masks()
    pari = np.zeros((P, P), np.float32)
    for c in range(P):
        pari[2 * (c % 64) + (c // 64), c] = 1.0
    pari = pari.astype(ml_dtypes.bfloat16)
    shared = dict(wqkvT=wqkvT, woT=woT, w1T=w1T, w2T=w2T, pari=pari)
    if has_bias:
        shared.update(
            bq=np.ascontiguousarray(bqkv[0:D].reshape(4, P).T),
            bk=np.ascontiguousarray(bqkv[D:2 * D].reshape(4, P).T),
            b1=np.ascontiguousarray(b1v.reshape(NHID, P).T),
            bv16=np.ascontiguousarray(
                (bqkv[2 * D:3 * D] * WS).astype(bf).reshape(1, D)),
            bo=bo, b2=b2)
    in_maps = []
    for c in range(NCORES):
        b_, i = divmod(c, 4)
        own = x[b_, i * CH:(i + 1) * CH]
        if i == 0:
            halo = np.zeros((HALO, D), f)
        else:
            halo = x[b_, i * CH - HALO:i * CH]
        xc = _pmaj(np.concatenate([halo, own], 0)).astype(bf)
        xo = _pmaj(own)
        in_maps.append(dict(xc=xc, xo=xo,
                            minv=(masks if i > 0 else masks0), **shared))
    return in_maps, has_bias


def kernel(**inputs):
    global LAST_EXEC_NS, LAST_RESULTS
    from concourse.bass_utils import run_bass_kernel_spmd

    in_maps, has_bias = make_in_maps(inputs)
    nc = get_nc(has_bias)
    trace = bool(int(os.environ.get("BASS_KERNEL_TRACE", "0")))
    res = run_bass_kernel_spmd(nc, in_maps, core_ids=list(range(NCORES)),
                               trace=trace)
    LAST_EXEC_NS = res.exec_time_ns
    LAST_RESULTS = res
    out = np.zeros((B, L, D), np.float32)
    for c, r in enumerate(res.results):
        b_, i = divmod(c, 4)
        out[b_, i * CH:(i + 1) * CH] = r["y"]
    return out
